# revision 20
# baseline (speedup 1.0000x reference)
"""Trainium2 Bass kernel for CropProposals (adaptive max-pool 2x2x2 over
data-dependent crops of a [4,128,24,24,24] feature map).

Design (v2, fold-pyramid):
  Each job (b,p) yields 8 octant regions of identical volume v.  Core k
  handles octant k of EVERY job, so all 8 cores have identical workload
  structure and run one uniform instruction stream (no Switch, no
  partition_id).  The host gathers, per core, each region (split into
  64-long windows if v>64, else padded to the next pow2 with duplicate
  in-region elements -- harmless for max) into a dense [C, N] bf16
  buffer grouped by pow2 level.  On-device, a fold pyramid of wide
  bf16 2x-mode tensor_tensor(max) instructions halves every level:
  64->32->...->2->1; level-h native data is DMA'd directly into its
  slot next to the fold outputs; vol-1 regions are DMA'd straight into
  the output row.  Split jobs get tiny per-np-class tensor_reduce
  combines over adjacent piece results.  Input DMA segments are issued
  from both HWDGE queues (sync + scalar) to overlap issue with
  transfer; the output row goes back as one small DMA.
"""

import numpy as np

_B, _C, _D, _H, _W = 4, 128, 24, 24, 24
_P = 64
_NCORES = 8
_SD, _SH = _H * _W, _W
_VOLF = _B * _D * _H * _W          # columns of the host-side [C, B*D*H*W] view

_LVLS = [512, 256, 128, 64, 32, 16, 8, 4, 2, 1]

_cache = {}


def _box_params(corners, scale):
    """Host-side replica of the reference bound math.

    Returns s, l, dlt arrays of shape [B, P, 3] (axis order D,H,W):
      region(o) along axis a = [ s + o*dlt , s + o*dlt + l )
    """
    c = np.asarray(corners).astype(np.int64)
    p1 = np.clip(c[:, :, 0, :] // scale, 0, 21)
    p2r = c[:, :, 1, :] // scale
    p2 = np.where(p2r - p1 >= 2, p2r, p1 + 2)
    sizes = np.array([_D, _H, _W], dtype=np.int64)
    e = np.minimum(p2, sizes)
    n = e - p1                 # crop length per axis, >= 2
    l = (n + 1) // 2           # region length (same for both regions)
    dlt = n // 2               # region-1 start offset from region-0 start
    return p1, l, dlt


def _octant_idx(b, sv, lv, dv, o):
    """Flat column indices (into [C, B*D*H*W]) of octant o of one job's
    region: [l1*l2*l3] in C-order."""
    ox, oy, oz = (o >> 2) & 1, (o >> 1) & 1, o & 1
    base = b * (_D * _H * _W)
    xs = (np.arange(sv[0], sv[0] + lv[0]) + ox * dv[0]) * _SD
    ys = (np.arange(sv[1], sv[1] + lv[1]) + oy * dv[1]) * _SH
    zs = np.arange(sv[2], sv[2] + lv[2]) + oz * dv[2]
    return (base + xs[:, None, None] + ys[None, :, None]
            + zs[None, None, :]).ravel()


def _pow2ceil(v):
    p = 1
    while p < v:
        p *= 2
    return p


class _Plan:
    """Static schedule derived from (corners, scale).  All cores share the
    identical level structure (each handles one octant of every job)."""

    def __init__(self, corners, scale):
        s, l, dlt = _box_params(corners, scale)
        vols = l.prod(axis=-1)                       # [B, P]

        # classify jobs.  Big jobs (v > 64) become np = ceil(v/64)
        # overlapping 64-windows; np <= 8 pads to a pow2 piece count with
        # duplicate windows and enters the pyramid as ONE tall unit at
        # level np'*64; np > 8 stays as L64 pieces + a combine reduce.
        jobs = []            # (jobid, b, p, v, level, [win_starts], split)
        for b in range(_B):
            for p in range(_P):
                v = int(vols[b, p])
                j = b * _P + p
                if v > 64:
                    npc = -(-v // 64)
                    starts = [min(i * 64, v - 64) for i in range(npc)]
                    npp = _pow2ceil(npc)
                    if npp <= 8:
                        starts = starts + [starts[-1]] * (npp - npc)
                        jobs.append((j, b, p, v, npp * 64, starts, False))
                    else:
                        jobs.append((j, b, p, v, 64, starts, True))
                else:
                    jobs.append((j, b, p, v, _pow2ceil(v), [0], False))
        self.jobs = jobs

        # per-level unit lists (shared ordering across cores)
        # unit = (jobid, [(win_start, win_len), ...]) with window lengths
        # summing to <= level (tail padded with dup elements when short)
        units = {h: [] for h in _LVLS}
        split_jobs = sorted([jb for jb in jobs if jb[6]],
                            key=lambda jb: (-len(jb[5]), jb[0]))
        for jb in split_jobs:
            for st in jb[5]:
                units[64].append((jb[0], [(st, 64)]))
        for jb in jobs:
            j, b, p, v, h, starts, split = jb
            if split:
                continue
            if v > 64:
                units[h].append((j, [(st, 64) for st in starts]))
            else:
                units[h].append((j, [(0, v)]))
        self.units = units
        n = {h: len(units[h]) for h in _LVLS}
        self.n = n

        # fold counts: F[h] = units folded into level h from the level above
        F = {_LVLS[0]: 0}
        N = {_LVLS[0]: n[_LVLS[0]]}
        for i in range(1, len(_LVLS)):
            h = _LVLS[i]
            F[h] = N[_LVLS[i - 1]]
            N[h] = F[h] + n[h]
        self.F, self.N = F, N

        # combine classes over split pieces (already grouped by np desc)
        combines = []        # (npc, m, piece_col0)  in level-1 col space
        i = 0
        pc = 0
        comb_jobs = []       # jobids in combine-output order
        while i < len(split_jobs):
            npc = len(split_jobs[i][5])
            m = 0
            while i + m < len(split_jobs) and len(split_jobs[i + m][5]) == npc:
                comb_jobs.append(split_jobs[i + m][0])
                m += 1
            combines.append((npc, m, pc))
            pc += npc * m
            i += m
        self.combines = combines
        ncomb = len(comb_jobs)

        # SBUF layout: level arrays with ALTERNATING orientation so the
        # native blocks of consecutive level pairs are contiguous (one DMA
        # segment per pair): even level-index -> [folded | native], odd ->
        # [native | folded].
        sb = {}
        off = 0
        for li, h in enumerate(_LVLS):
            fold_sz = F[h] * h
            nat_sz = n[h] * h
            fn = (li % 2 == 0)                       # [folded | native]
            pad = (fold_sz & 1) if fn else (nat_sz & 1)
            if fn:
                sb[h] = {"base": off, "fold": off,
                         "nat": off + fold_sz + pad, "fn": True}
            else:
                sb[h] = {"base": off, "nat": off,
                         "fold": off + nat_sz + pad, "fn": False}
            sb[h]["pad"] = pad
            off += fold_sz + nat_sz + pad
        self.out_base = sb[1]["base"]                # A1 = output row start
        self.comb_base = off                         # combine outputs
        off += ncomb
        off += off & 1
        self.sbuf_cols = off
        self.sb = sb
        self.out_cols = off - self.out_base

        # DRAM layout: native blocks packed in level order (contiguous with
        # the SBUF native blocks pair by pair)
        dram = {}
        doff = 0
        for h in _LVLS:
            dram[h] = doff
            doff += n[h] * h
        self.dram = dram
        self.dram_cols = doff

        # input DMA segments: one per level PAIR (native blocks contiguous
        # by the alternating orientation); the {128,64} pair is split into
        # two chunks at a 64-unit boundary so fold-64 can chase the DMA.
        # (engine, dram_off, sbuf_off, cols, set_of_levels)
        pairs = [(_LVLS[i], _LVLS[i + 1]) for i in range(0, len(_LVLS), 2)]
        engines = ["scalar", "sync", "scalar", "scalar", "sync"]
        segs = []
        self.ua_units = 0
        for (a, b), eng in zip(pairs, engines):
            cols = n[a] * a + n[b] * b
            if cols == 0:
                continue
            if n[a]:
                assert sb[a]["nat"] + n[a] * a == sb[b]["nat"] or not n[b]
                assert dram[a] + n[a] * a == dram[b]
            so, do = (sb[a]["nat"], dram[a]) if n[a] else \
                     (sb[b]["nat"], dram[b])
            if (a, b) == (128, 64) and n[64] > 2:
                ua_units = max(1, n[64] * 11 // 20)
                self.ua_units = ua_units
                cut = n[a] * a + ua_units * 64
                segs.append(("sync", do, so, cut, {a, "64a"}))
                segs.append(("sync", do + cut, so + cut, cols - cut,
                             {"64b"}))
            else:
                segs.append((eng, do, so, cols, {a, b}))
        self.segs = segs

        # host output mapping: job -> col within the output row.  A unit's
        # position within its level array propagates unchanged through the
        # folds except for the native-block shift at [nat | folded] levels.
        self.outcol = {}
        for h in _LVLS:
            for i, (j, wins) in enumerate(units[h]):
                if not jobs[j][6]:
                    self.outcol[j] = self._lvl1col(h, i)
        for q, j in enumerate(comb_jobs):
            self.outcol[j] = (self.comb_base - self.out_base) + q
        self.split_l1c0 = self._lvl1col(64, 0) if combines else 0

        # gather indices per core (dram image column -> fmT column)
        self.core_idx = []
        for k in range(_NCORES):
            parts = []
            for h in _LVLS:
                for (j, wins) in units[h]:
                    jb = jobs[j]
                    b, p = jb[1], jb[2]
                    sv = [int(x) for x in s[b, p]]
                    lv = [int(x) for x in l[b, p]]
                    dv = [int(x) for x in dlt[b, p]]
                    full = _octant_idx(b, sv, lv, dv, k)
                    w = np.concatenate([full[st:st + ln]
                                        for (st, ln) in wins])
                    if w.size < h:                   # pad with dup elements
                        w = np.concatenate(
                            [w, np.full(h - w.size, w[0], np.int64)])
                    assert w.size == h
                    parts.append(w)
            idx = np.concatenate(parts) if parts else np.zeros(0, np.int64)
            assert idx.size == self.dram_cols, (idx.size, self.dram_cols)
            self.core_idx.append(idx)

    def _lvl1col(self, h, i):
        """Level-1 output column of native unit #i of level h."""
        pos = i + (self.F[h] if self.sb[h]["fn"] else 0)
        for g in _LVLS[_LVLS.index(h) + 1:]:
            if not self.sb[g]["fn"]:                 # [nat | folded]
                pos += self.n[g] + (self.sb[g]["pad"] if g == 1 else 0)
        return pos


def _build_program(plan):
    """Raw Bacc build: one uniform program for all cores."""
    import concourse.bacc as bacc
    import concourse.bass as bass_mod
    import concourse.mybir as mybir
    from concourse.ap import AP

    orig_memset = bass_mod.BassGpSimd.memset
    orig_barrier = bass_mod.Bass.all_engine_barrier
    bass_mod.BassGpSimd.memset = lambda self, ap, c: None
    bass_mod.Bass.all_engine_barrier = lambda self, **kw: None
    try:
        nc = bacc.Bacc("TRN2", target_bir_lowering=False, debug=False,
                       num_devices=_NCORES)
    finally:
        bass_mod.BassGpSimd.memset = orig_memset
        bass_mod.Bass.all_engine_barrier = orig_barrier

    x_in = nc.dram_tensor("fm", [_C, plan.dram_cols], mybir.dt.bfloat16,
                          kind="ExternalInput")
    y_out = nc.dram_tensor("out", [_C, plan.out_cols], mybir.dt.bfloat16,
                           kind="ExternalOutput")

    from contextlib import ExitStack
    with ExitStack() as stk:
        xt = stk.enter_context(
            nc.sbuf_tensor("xt", [_C, plan.sbuf_cols], mybir.dt.bfloat16))
        seg_sems = [stk.enter_context(nc.semaphore(f"seg{i}"))
                    for i in range(len(plan.segs))]
        v_sem = stk.enter_context(nc.semaphore("v_sem"))
        out_sem = stk.enter_context(nc.semaphore("out_sem"))
        block = stk.enter_context(nc.Block())

        sync_segs = [i for i, sg in enumerate(plan.segs) if sg[0] == "sync"]
        scal_segs = [i for i, sg in enumerate(plan.segs) if sg[0] == "scalar"]
        seg_by_level = {}
        for i, sg in enumerate(plan.segs):
            for lv in sg[4]:
                seg_by_level.setdefault(lv, []).append(i)

        @block.sync
        def _(sync):
            for i in sync_segs:
                _, do, so, cols, _h = plan.segs[i]
                sync.dma_start(out=xt[:, so:so + cols],
                               in_=x_in[:, do:do + cols]
                               ).then_inc(seg_sems[i], 16)
            # out DMA: v_sem implies vector done AND nat1 landed
            sync.wait_ge(v_sem, 1)
            sync.dma_start(out=y_out[:, :],
                           in_=xt[:, plan.out_base:
                                  plan.out_base + plan.out_cols]
                           ).then_inc(out_sem, 16)

        @block.scalar
        def _(scalar):
            for i in scal_segs:
                _, do, so, cols, _h = plan.segs[i]
                scalar.dma_start(out=xt[:, so:so + cols],
                                 in_=x_in[:, do:do + cols]
                                 ).then_inc(seg_sems[i], 16)

        @block.vector
        def _(vector):
            base = xt[:]
            part = list(base.ap[0])
            t, o0 = base.tensor, base.offset
            sb, F, N, n = plan.sb, plan.F, plan.N, plan.n

            def fold(in_off, h, cnt, out_off):
                h2 = h // 2
                in0 = AP(t, o0 + in_off, [part, [h, cnt], [1, h2]])
                in1 = AP(t, o0 + in_off + h2, [part, [h, cnt], [1, h2]])
                out = AP(t, o0 + out_off, [part, [h2, cnt], [1, h2]])
                return vector.tensor_tensor(out=out, in0=in0, in1=in1,
                                            op=mybir.AluOpType.max)

            waited = set()

            def wait_segs(ids):
                for i in ids:
                    if i not in waited:
                        vector.wait_ge(seg_sems[i], 16)
                        waited.add(i)

            # fold pyramid top -> bottom
            for li, h in enumerate(_LVLS[:-1]):
                nxt = _LVLS[li + 1]
                if N[h] == 0:
                    continue
                if h == 64 and plan.ua_units and n[64] > plan.ua_units:
                    # chase the two {128,64}-pair sub-chunk DMAs; natives
                    # lead the array ([nat | folded] orientation)
                    ua = plan.ua_units
                    wait_segs(seg_by_level.get("64a", []))
                    fold(sb[h]["base"], h, ua, sb[nxt]["fold"])
                    wait_segs(seg_by_level.get("64b", []))
                    fold(sb[h]["base"] + ua * h, h, N[h] - ua,
                         sb[nxt]["fold"] + ua * (h // 2))
                else:
                    if n[h]:
                        wait_segs(seg_by_level.get(h, []))
                    fold(sb[h]["base"], h, N[h], sb[nxt]["fold"])

            # combines over split piece results in the output row
            # (fold-2 already waited the {2,1} segment, so nat1 landed)
            last = None
            a1 = plan.out_base
            ccol = plan.comb_base
            for (npc, m, pc0) in plan.combines:
                in_ = AP(t, o0 + a1 + plan.split_l1c0 + pc0,
                         [part, [npc, m], [1, npc]])
                out = AP(t, o0 + ccol, [part, [1, m]])
                last = vector.tensor_reduce(out=out, in_=in_,
                                            axis=mybir.AxisListType.X,
                                            op=mybir.AluOpType.max)
                ccol += m
            fin = last if last is not None else vector.engine_nop()
            fin.then_inc(v_sem, 1)

    nc.compile()
    return nc


def _get_program(corners, scale):
    key = (np.asarray(corners).tobytes(), int(scale))
    if key not in _cache:
        plan = _Plan(corners, scale)
        nc = _build_program(plan)
        _cache[key] = (nc, plan)
    return _cache[key]


def _install_ntff_shim():
    """The agent image's antenv lacks axon_hooks; recreate it so
    run_bass_kernel_spmd(trace=True) can capture NTFF profiles."""
    import sys
    import types
    try:
        import antenv.axon_hooks  # noqa: F401
        return
    except ImportError:
        pass
    try:
        from trn_agent_boot.trn_boot import _ntff_profile_via_ctypes
        hook = _ntff_profile_via_ctypes("/opt/axon/libaxon_pjrt.so")
        mod = types.ModuleType("antenv.axon_hooks")
        mod._hook = hook
        mod.get_axon_ntff_profile_hook = lambda: mod._hook

        def _set(h):
            mod._hook = h

        mod.set_axon_ntff_profile_hook = _set
        sys.modules["antenv.axon_hooks"] = mod
        import antenv
        antenv.axon_hooks = mod
    except Exception:
        pass


def _run(fm, corners, scale, trace=False, trace_cores=None):
    from concourse.bass_utils import run_bass_kernel_spmd
    import ml_dtypes
    if trace:
        _install_ntff_shim()

    fm = np.asarray(fm, dtype=np.float32)
    scale = int(scale)
    nc, plan = _get_program(corners, scale)

    fmT = np.ascontiguousarray(fm.transpose(1, 0, 2, 3, 4)).reshape(_C, _VOLF)
    fmT16 = fmT.astype(ml_dtypes.bfloat16)
    in_maps = []
    for k in range(_NCORES):
        in_maps.append(
            {"fm": np.ascontiguousarray(fmT16[:, plan.core_idx[k]])})

    kwargs = {}
    if trace:
        kwargs.update(trace=True,
                      trace_cores=trace_cores or list(range(_NCORES)))
    res = run_bass_kernel_spmd(nc, in_maps, list(range(_NCORES)), **kwargs)

    ys = np.stack([np.asarray(res.results[k]["out"]).astype(np.float32)
                   for k in range(_NCORES)])          # [8, C, out_cols]
    cols = np.array([plan.outcol[j] for j in range(_B * _P)])
    g = ys[:, :, cols]                                # [8, C, B*P]
    out = np.ascontiguousarray(
        g.transpose(2, 1, 0)).reshape(_B, _P, _C, 2, 2, 2)
    return out, getattr(res, "exec_time_ns", None)


def kernel(fm, corners, scale=4):
    out, _ = _run(fm, corners, scale, trace=False)
    return out


# revision 21
# speedup vs baseline: 1.0901x; 1.0901x over previous
"""Trainium2 Bass kernel for CropProposals (adaptive max-pool 2x2x2 over
data-dependent crops of a [4,128,24,24,24] feature map).

Design (v2, fold-pyramid):
  Each job (b,p) yields 8 octant regions of identical volume v.  Core k
  handles octant k of EVERY job, so all 8 cores have identical workload
  structure and run one uniform instruction stream (no Switch, no
  partition_id).  The host gathers, per core, each region (split into
  64-long windows if v>64, else padded to the next pow2 with duplicate
  in-region elements -- harmless for max) into a dense [C, N] bf16
  buffer grouped by pow2 level.  On-device, a fold pyramid of wide
  bf16 2x-mode tensor_tensor(max) instructions halves every level:
  64->32->...->2->1; level-h native data is DMA'd directly into its
  slot next to the fold outputs; vol-1 regions are DMA'd straight into
  the output row.  Split jobs get tiny per-np-class tensor_reduce
  combines over adjacent piece results.  Input DMA segments are issued
  from both HWDGE queues (sync + scalar) to overlap issue with
  transfer; the output row goes back as one small DMA.
"""

import numpy as np

_B, _C, _D, _H, _W = 4, 128, 24, 24, 24
_P = 64
_NCORES = 8
_SD, _SH = _H * _W, _W
_VOLF = _B * _D * _H * _W          # columns of the host-side [C, B*D*H*W] view

_LVLS = [512, 256, 128, 64, 32, 16, 8, 4, 2, 1]

_cache = {}


def _box_params(corners, scale):
    """Host-side replica of the reference bound math.

    Returns s, l, dlt arrays of shape [B, P, 3] (axis order D,H,W):
      region(o) along axis a = [ s + o*dlt , s + o*dlt + l )
    """
    c = np.asarray(corners).astype(np.int64)
    p1 = np.clip(c[:, :, 0, :] // scale, 0, 21)
    p2r = c[:, :, 1, :] // scale
    p2 = np.where(p2r - p1 >= 2, p2r, p1 + 2)
    sizes = np.array([_D, _H, _W], dtype=np.int64)
    e = np.minimum(p2, sizes)
    n = e - p1                 # crop length per axis, >= 2
    l = (n + 1) // 2           # region length (same for both regions)
    dlt = n // 2               # region-1 start offset from region-0 start
    return p1, l, dlt


def _octant_idx(b, sv, lv, dv, o):
    """Flat column indices (into [C, B*D*H*W]) of octant o of one job's
    region: [l1*l2*l3] in C-order."""
    ox, oy, oz = (o >> 2) & 1, (o >> 1) & 1, o & 1
    base = b * (_D * _H * _W)
    xs = (np.arange(sv[0], sv[0] + lv[0]) + ox * dv[0]) * _SD
    ys = (np.arange(sv[1], sv[1] + lv[1]) + oy * dv[1]) * _SH
    zs = np.arange(sv[2], sv[2] + lv[2]) + oz * dv[2]
    return (base + xs[:, None, None] + ys[None, :, None]
            + zs[None, None, :]).ravel()


def _pow2ceil(v):
    p = 1
    while p < v:
        p *= 2
    return p


class _Plan:
    """Static schedule derived from (corners, scale).  All cores share the
    identical level structure (each handles one octant of every job)."""

    def __init__(self, corners, scale):
        s, l, dlt = _box_params(corners, scale)
        vols = l.prod(axis=-1)                       # [B, P]

        # classify jobs.  Big jobs (v > 64) become np = ceil(v/64)
        # overlapping 64-windows; np <= 8 pads to a pow2 piece count with
        # duplicate windows and enters the pyramid as ONE tall unit at
        # level np'*64; np > 8 stays as L64 pieces + a combine reduce.
        jobs = []            # (jobid, b, p, v, level, [win_starts], split)
        for b in range(_B):
            for p in range(_P):
                v = int(vols[b, p])
                j = b * _P + p
                if v > 64:
                    npc = -(-v // 64)
                    starts = [min(i * 64, v - 64) for i in range(npc)]
                    npp = _pow2ceil(npc)
                    if npp <= 8:
                        starts = starts + [starts[-1]] * (npp - npc)
                        jobs.append((j, b, p, v, npp * 64, starts, False))
                    else:
                        jobs.append((j, b, p, v, 64, starts, True))
                else:
                    # min level 4: keeps every DMA segment's per-partition
                    # descriptor >= 512B and drops the tail {2,1} segment
                    jobs.append((j, b, p, v, max(4, _pow2ceil(v)),
                                 [0], False))
        self.jobs = jobs

        # per-level unit lists (shared ordering across cores)
        # unit = (jobid, [(win_start, win_len), ...]) with window lengths
        # summing to <= level (tail padded with dup elements when short)
        units = {h: [] for h in _LVLS}
        split_jobs = sorted([jb for jb in jobs if jb[6]],
                            key=lambda jb: (-len(jb[5]), jb[0]))
        for jb in split_jobs:
            for st in jb[5]:
                units[64].append((jb[0], [(st, 64)]))
        for jb in jobs:
            j, b, p, v, h, starts, split = jb
            if split:
                continue
            if v > 64:
                units[h].append((j, [(st, 64) for st in starts]))
            else:
                units[h].append((j, [(0, v)]))
        self.units = units
        n = {h: len(units[h]) for h in _LVLS}
        self.n = n

        # fold counts: F[h] = units folded into level h from the level above
        F = {_LVLS[0]: 0}
        N = {_LVLS[0]: n[_LVLS[0]]}
        for i in range(1, len(_LVLS)):
            h = _LVLS[i]
            F[h] = N[_LVLS[i - 1]]
            N[h] = F[h] + n[h]
        self.F, self.N = F, N

        # combine classes over split pieces (already grouped by np desc)
        combines = []        # (npc, m, piece_col0)  in level-1 col space
        i = 0
        pc = 0
        comb_jobs = []       # jobids in combine-output order
        while i < len(split_jobs):
            npc = len(split_jobs[i][5])
            m = 0
            while i + m < len(split_jobs) and len(split_jobs[i + m][5]) == npc:
                comb_jobs.append(split_jobs[i + m][0])
                m += 1
            combines.append((npc, m, pc))
            pc += npc * m
            i += m
        self.combines = combines
        ncomb = len(comb_jobs)

        # SBUF layout: level arrays with ALTERNATING orientation so the
        # native blocks of consecutive level pairs are contiguous (one DMA
        # segment per pair): even level-index -> [folded | native], odd ->
        # [native | folded].
        sb = {}
        off = 0
        for li, h in enumerate(_LVLS):
            fold_sz = F[h] * h
            nat_sz = n[h] * h
            fn = (li % 2 == 0)                       # [folded | native]
            pad = (fold_sz & 1) if fn else (nat_sz & 1)
            if fn:
                sb[h] = {"base": off, "fold": off,
                         "nat": off + fold_sz + pad, "fn": True}
            else:
                sb[h] = {"base": off, "nat": off,
                         "fold": off + nat_sz + pad, "fn": False}
            sb[h]["pad"] = pad
            off += fold_sz + nat_sz + pad
        self.out_base = sb[1]["base"]                # A1 = output row start
        self.comb_base = off                         # combine outputs
        off += ncomb
        off += off & 1
        self.sbuf_cols = off
        self.sb = sb
        self.out_cols = off - self.out_base

        # DRAM layout: native blocks packed in level order (contiguous with
        # the SBUF native blocks pair by pair)
        dram = {}
        doff = 0
        for h in _LVLS:
            dram[h] = doff
            doff += n[h] * h
        self.dram = dram
        self.dram_cols = doff

        # input DMA segments: one per level PAIR (native blocks contiguous
        # by the alternating orientation); the {128,64} pair is split into
        # two chunks at a 64-unit boundary so fold-64 can chase the DMA.
        # (engine, dram_off, sbuf_off, cols, set_of_levels)
        pairs = [(_LVLS[i], _LVLS[i + 1]) for i in range(0, len(_LVLS), 2)]
        engines = ["scalar", "sync", "scalar", "scalar", "sync"]
        segs = []
        self.ua_units = 0
        for (a, b), eng in zip(pairs, engines):
            cols = n[a] * a + n[b] * b
            if cols == 0:
                continue
            if n[a]:
                assert sb[a]["nat"] + n[a] * a == sb[b]["nat"] or not n[b]
                assert dram[a] + n[a] * a == dram[b]
            so, do = (sb[a]["nat"], dram[a]) if n[a] else \
                     (sb[b]["nat"], dram[b])
            if (a, b) == (128, 64) and n[64] > 2:
                ua_units = max(1, n[64] * 11 // 20)
                self.ua_units = ua_units
                cut = n[a] * a + ua_units * 64
                segs.append(("sync", do, so, cut, {a, "64a"}))
                segs.append(("sync", do + cut, so + cut, cols - cut,
                             {"64b"}))
            else:
                segs.append((eng, do, so, cols, {a, b}))
        self.segs = segs

        # host output mapping: job -> col within the output row.  A unit's
        # position within its level array propagates unchanged through the
        # folds except for the native-block shift at [nat | folded] levels.
        self.outcol = {}
        for h in _LVLS:
            for i, (j, wins) in enumerate(units[h]):
                if not jobs[j][6]:
                    self.outcol[j] = self._lvl1col(h, i)
        for q, j in enumerate(comb_jobs):
            self.outcol[j] = (self.comb_base - self.out_base) + q
        self.split_l1c0 = self._lvl1col(64, 0) if combines else 0

        # gather indices per core (dram image column -> fmT column)
        self.core_idx = []
        for k in range(_NCORES):
            parts = []
            for h in _LVLS:
                for (j, wins) in units[h]:
                    jb = jobs[j]
                    b, p = jb[1], jb[2]
                    sv = [int(x) for x in s[b, p]]
                    lv = [int(x) for x in l[b, p]]
                    dv = [int(x) for x in dlt[b, p]]
                    full = _octant_idx(b, sv, lv, dv, k)
                    w = np.concatenate([full[st:st + ln]
                                        for (st, ln) in wins])
                    if w.size < h:                   # pad with dup elements
                        w = np.concatenate(
                            [w, np.full(h - w.size, w[0], np.int64)])
                    assert w.size == h
                    parts.append(w)
            idx = np.concatenate(parts) if parts else np.zeros(0, np.int64)
            assert idx.size == self.dram_cols, (idx.size, self.dram_cols)
            self.core_idx.append(idx)

    def _lvl1col(self, h, i):
        """Level-1 output column of native unit #i of level h."""
        pos = i + (self.F[h] if self.sb[h]["fn"] else 0)
        for g in _LVLS[_LVLS.index(h) + 1:]:
            if not self.sb[g]["fn"]:                 # [nat | folded]
                pos += self.n[g] + (self.sb[g]["pad"] if g == 1 else 0)
        return pos


def _build_program(plan):
    """Raw Bacc build: one uniform program for all cores."""
    import concourse.bacc as bacc
    import concourse.bass as bass_mod
    import concourse.mybir as mybir
    from concourse.ap import AP

    orig_memset = bass_mod.BassGpSimd.memset
    orig_barrier = bass_mod.Bass.all_engine_barrier
    bass_mod.BassGpSimd.memset = lambda self, ap, c: None
    bass_mod.Bass.all_engine_barrier = lambda self, **kw: None
    try:
        nc = bacc.Bacc("TRN2", target_bir_lowering=False, debug=False,
                       num_devices=_NCORES)
    finally:
        bass_mod.BassGpSimd.memset = orig_memset
        bass_mod.Bass.all_engine_barrier = orig_barrier

    x_in = nc.dram_tensor("fm", [_C, plan.dram_cols], mybir.dt.bfloat16,
                          kind="ExternalInput")
    y_out = nc.dram_tensor("out", [_C, plan.out_cols], mybir.dt.bfloat16,
                           kind="ExternalOutput")

    from contextlib import ExitStack
    with ExitStack() as stk:
        xt = stk.enter_context(
            nc.sbuf_tensor("xt", [_C, plan.sbuf_cols], mybir.dt.bfloat16))
        seg_sems = [stk.enter_context(nc.semaphore(f"seg{i}"))
                    for i in range(len(plan.segs))]
        v_sem = stk.enter_context(nc.semaphore("v_sem"))
        out_sem = stk.enter_context(nc.semaphore("out_sem"))
        block = stk.enter_context(nc.Block())

        sync_segs = [i for i, sg in enumerate(plan.segs) if sg[0] == "sync"]
        scal_segs = [i for i, sg in enumerate(plan.segs) if sg[0] == "scalar"]
        seg_by_level = {}
        for i, sg in enumerate(plan.segs):
            for lv in sg[4]:
                seg_by_level.setdefault(lv, []).append(i)

        @block.sync
        def _(sync):
            for i in sync_segs:
                _, do, so, cols, _h = plan.segs[i]
                sync.dma_start(out=xt[:, so:so + cols],
                               in_=x_in[:, do:do + cols]
                               ).then_inc(seg_sems[i], 16)
            # out DMA: v_sem implies vector done AND nat1 landed
            sync.wait_ge(v_sem, 1)
            sync.dma_start(out=y_out[:, :],
                           in_=xt[:, plan.out_base:
                                  plan.out_base + plan.out_cols]
                           ).then_inc(out_sem, 16)

        @block.scalar
        def _(scalar):
            for i in scal_segs:
                _, do, so, cols, _h = plan.segs[i]
                scalar.dma_start(out=xt[:, so:so + cols],
                                 in_=x_in[:, do:do + cols]
                                 ).then_inc(seg_sems[i], 16)

        @block.vector
        def _(vector):
            base = xt[:]
            part = list(base.ap[0])
            t, o0 = base.tensor, base.offset
            sb, F, N, n = plan.sb, plan.F, plan.N, plan.n

            def fold(in_off, h, cnt, out_off):
                h2 = h // 2
                in0 = AP(t, o0 + in_off, [part, [h, cnt], [1, h2]])
                in1 = AP(t, o0 + in_off + h2, [part, [h, cnt], [1, h2]])
                out = AP(t, o0 + out_off, [part, [h2, cnt], [1, h2]])
                return vector.tensor_tensor(out=out, in0=in0, in1=in1,
                                            op=mybir.AluOpType.max)

            waited = set()

            def wait_segs(ids):
                for i in ids:
                    if i not in waited:
                        vector.wait_ge(seg_sems[i], 16)
                        waited.add(i)

            # fold pyramid top -> bottom
            for li, h in enumerate(_LVLS[:-1]):
                nxt = _LVLS[li + 1]
                if N[h] == 0:
                    continue
                if h == 64 and plan.ua_units and n[64] > plan.ua_units:
                    # chase the two {128,64}-pair sub-chunk DMAs; natives
                    # lead the array ([nat | folded] orientation)
                    ua = plan.ua_units
                    wait_segs(seg_by_level.get("64a", []))
                    fold(sb[h]["base"], h, ua, sb[nxt]["fold"])
                    wait_segs(seg_by_level.get("64b", []))
                    fold(sb[h]["base"] + ua * h, h, N[h] - ua,
                         sb[nxt]["fold"] + ua * (h // 2))
                else:
                    if n[h]:
                        wait_segs(seg_by_level.get(h, []))
                    fold(sb[h]["base"], h, N[h], sb[nxt]["fold"])

            # combines over split piece results in the output row
            # (fold-2 already waited the {2,1} segment, so nat1 landed)
            last = None
            a1 = plan.out_base
            ccol = plan.comb_base
            for (npc, m, pc0) in plan.combines:
                in_ = AP(t, o0 + a1 + plan.split_l1c0 + pc0,
                         [part, [npc, m], [1, npc]])
                out = AP(t, o0 + ccol, [part, [1, m]])
                last = vector.tensor_reduce(out=out, in_=in_,
                                            axis=mybir.AxisListType.X,
                                            op=mybir.AluOpType.max)
                ccol += m
            fin = last if last is not None else vector.engine_nop()
            fin.then_inc(v_sem, 1)

    nc.compile()
    return nc


def _get_program(corners, scale):
    key = (np.asarray(corners).tobytes(), int(scale))
    if key not in _cache:
        plan = _Plan(corners, scale)
        nc = _build_program(plan)
        _cache[key] = (nc, plan)
    return _cache[key]


def _install_ntff_shim():
    """The agent image's antenv lacks axon_hooks; recreate it so
    run_bass_kernel_spmd(trace=True) can capture NTFF profiles."""
    import sys
    import types
    try:
        import antenv.axon_hooks  # noqa: F401
        return
    except ImportError:
        pass
    try:
        from trn_agent_boot.trn_boot import _ntff_profile_via_ctypes
        hook = _ntff_profile_via_ctypes("/opt/axon/libaxon_pjrt.so")
        mod = types.ModuleType("antenv.axon_hooks")
        mod._hook = hook
        mod.get_axon_ntff_profile_hook = lambda: mod._hook

        def _set(h):
            mod._hook = h

        mod.set_axon_ntff_profile_hook = _set
        sys.modules["antenv.axon_hooks"] = mod
        import antenv
        antenv.axon_hooks = mod
    except Exception:
        pass


def _run(fm, corners, scale, trace=False, trace_cores=None):
    from concourse.bass_utils import run_bass_kernel_spmd
    import ml_dtypes
    if trace:
        _install_ntff_shim()

    fm = np.asarray(fm, dtype=np.float32)
    scale = int(scale)
    nc, plan = _get_program(corners, scale)

    fmT = np.ascontiguousarray(fm.transpose(1, 0, 2, 3, 4)).reshape(_C, _VOLF)
    fmT16 = fmT.astype(ml_dtypes.bfloat16)
    in_maps = []
    for k in range(_NCORES):
        in_maps.append(
            {"fm": np.ascontiguousarray(fmT16[:, plan.core_idx[k]])})

    kwargs = {}
    if trace:
        kwargs.update(trace=True,
                      trace_cores=trace_cores or list(range(_NCORES)))
    res = run_bass_kernel_spmd(nc, in_maps, list(range(_NCORES)), **kwargs)

    ys = np.stack([np.asarray(res.results[k]["out"]).astype(np.float32)
                   for k in range(_NCORES)])          # [8, C, out_cols]
    cols = np.array([plan.outcol[j] for j in range(_B * _P)])
    g = ys[:, :, cols]                                # [8, C, B*P]
    out = np.ascontiguousarray(
        g.transpose(2, 1, 0)).reshape(_B, _P, _C, 2, 2, 2)
    return out, getattr(res, "exec_time_ns", None)


def kernel(fm, corners, scale=4):
    out, _ = _run(fm, corners, scale, trace=False)
    return out
